# revision 2
# baseline (speedup 1.0000x reference)
"""Trainium2 Bass kernel for nn_NonSOCSymmetricContraction.

Math (reference):
  wy_o = einsum('ekqa,be->bkqa', w_o, y)             o in {1,2,3}
  t1[b,a] = sum_p coeff1[p] * x[b,a,i1,l1] * wy1[b,k1,q1,a]
  t2[b,a] = sum_p coeff2[p] * x[b,a,i2,l2] * x[b,a,j2,m2] * wy2[b,k2,q2,a]
  t3[b,a] = sum_p coeff3[p] * x[b,a,i3,l3] * x[b,a,j3,m3] * x[b,a,f3,g3] * wy3[b,k3,q3,a]
  out = t1 + t2 + t3                                  [B, A]

Device algorithm (per core, data-parallel over B; BL = B/8 = 512).
All matmul operands are bf16.  x is stored with even a's on partitions
0-63 and odd a's on 64-127, so an (even, odd) a-pair shares one
BL-column block and its gathers run on disjoint PE row groups.

v2 schedule: pairs are processed two at a time (a "group" = 4 a's =
one c4 accumulator).  Stages per group: p3 h=0, p3 h=1, p2+finalize,
each stage covering BOTH pairs so the M=32 contract matmuls of the two
pairs land on all four PE column groups back-to-back (4-way col
tiling), and M=64 folded matmuls 2-way tile across pairs.

Engine balance: the path3 product chain per (pair, h) is assigned one
of two shapes to spread the element-wise load:
  type A: ACT copy gb->SBUF;       DVE t = ga*gb_sb; DVE x3 = gc*t
  type C: ACT copy gb, gc->SBUF;   DVE t = ga*gb_sb; GPSIMD x3 = gc_sb*t
xsq (for the path2 square-trick correction) is computed on GpSimd in
per-pair [128, BL] chunks, interleaved with the type-C products.
Finalize per group: v = c4*y4 (DVE), sig block-sum matmul writes into
rows 0:4 of the (just consumed) c4 PSUM bank, ACT copies it to SBUF,
DMA out.  c4 is double-buffered (psum_c bufs=2) so the next group's
contracts never wait on the previous finalize.
"""

import sys

import numpy as np

if "/opt/trn_rl_repo" not in sys.path:
    sys.path.insert(0, "/opt/trn_rl_repo")

B, A, L, M, E = 4096, 64, 16, 4, 10
NCORES = 8
BL = B // NCORES  # 512
P1, P2, P3 = 32, 128, 256
AG = 4  # a-values packed per PSUM accumulator
NG = A // AG  # 16 a-groups

# number of path3 (pair, h) units whose second product runs on GpSimd
NGPS = 30

_CACHE: dict = {}


def _build_module():
    """Build and compile the (input-independent) Bass module once."""
    import concourse.bacc as bacc
    import concourse.mybir as mybir
    from concourse import tile

    f32 = mybir.dt.float32
    bf16 = mybir.dt.bfloat16

    nc = bacc.Bacc(
        "TRN2",
        target_bir_lowering=False,
        debug=False,
        enable_asserts=False,
        num_devices=NCORES,
    )

    XW = (A // 2) * BL  # 16384
    xt_d = nc.dram_tensor("xt", [128, XW], bf16, kind="ExternalInput")
    y4_d = nc.dram_tensor("y4", [128, BL], bf16, kind="ExternalInput")
    sel2_d = nc.dram_tensor("sel2", [128, 128], bf16, kind="ExternalInput")
    sel3_d = nc.dram_tensor("sel3", [128, 768], bf16, kind="ExternalInput")
    a1w_d = nc.dram_tensor("a1w", [128, A * 32], bf16, kind="ExternalInput")
    sqw_d = nc.dram_tensor("sqw", [128, A * 32], bf16, kind="ExternalInput")
    wg2_d = nc.dram_tensor("wg2", [128, A * 32], bf16, kind="ExternalInput")
    wg3_d = nc.dram_tensor("wg3", [128, A * 64], bf16, kind="ExternalInput")
    sig_d = nc.dram_tensor("sig", [128, 4], bf16, kind="ExternalInput")
    out_d = nc.dram_tensor("out", [AG, NG * BL], f32, kind="ExternalOutput")

    with tile.TileContext(nc) as tc:
        with (
            tc.tile_pool(name="const", bufs=1) as const,
            tc.tile_pool(name="work", bufs=2) as work,
            tc.tile_pool(name="psum_g", bufs=3, space="PSUM") as psum_g,
            tc.tile_pool(name="psum_c", bufs=2, space="PSUM") as psum_c,
        ):
            # Tile-pool dependencies are tile-granular: a consumer of xt
            # waits for ALL DMAs into that tile. Split xt into 4 tiles
            # (8 a-pairs each) so pair-0 compute starts after the first
            # quarter lands instead of the full 4MB input load.
            NXT = 4
            XQ = XW // NXT  # columns per xt tile (8 pair-blocks)
            xt_tiles = []
            xt0 = const.tile([128, XQ], bf16, name="xt0")
            xt_tiles.append(xt0)
            for c in range(2):
                nc.sync.dma_start(
                    out=xt0[:, c * XQ // 2 : (c + 1) * XQ // 2],
                    in_=xt_d[:, c * XQ // 2 : (c + 1) * XQ // 2],
                )
            sel2 = const.tile([128, 128], bf16)
            nc.sync.dma_start(out=sel2[:], in_=sel2_d[:])
            sel3 = const.tile([128, 768], bf16)
            nc.sync.dma_start(out=sel3[:], in_=sel3_d[:])
            a1w = const.tile([128, A * 32], bf16)
            nc.sync.dma_start(out=a1w[:], in_=a1w_d[:])
            sqw = const.tile([128, A * 32], bf16)
            nc.sync.dma_start(out=sqw[:], in_=sqw_d[:])
            wg2 = const.tile([128, A * 32], bf16)
            nc.sync.dma_start(out=wg2[:], in_=wg2_d[:])
            wg3 = const.tile([128, A * 64], bf16)
            nc.sync.dma_start(out=wg3[:], in_=wg3_d[:])
            y4 = const.tile([128, BL], bf16)
            nc.sync.dma_start(out=y4[:], in_=y4_d[:])
            sig = const.tile([128, 4], bf16)
            nc.sync.dma_start(out=sig[:], in_=sig_d[:])
            for t in range(1, NXT):
                xti = const.tile([128, XQ], bf16, name=f"xt{t}")
                xt_tiles.append(xti)
                for c in range(2):
                    nc.sync.dma_start(
                        out=xti[:, c * XQ // 2 : (c + 1) * XQ // 2],
                        in_=xt_d[:, t * XQ + c * XQ // 2
                                 : t * XQ + (c + 1) * XQ // 2],
                    )

            # xsq = xt * xt on GpSimd, emitted lazily in per-pair
            # [128, BL] chunks (interleaved with type-C products so the
            # first pairs' sqw contracts aren't stuck behind 30us of
            # squaring).
            xsq_tiles = []
            for t in range(NXT):
                xsqi = const.tile([128, XQ], bf16, name=f"xsq{t}")
                xsq_tiles.append(xsqi)

            PPT = (A // 2) // NXT  # a-pairs per xt tile

            def xt_of(ap):
                return xt_tiles[ap // PPT], (ap % PPT) * BL

            def xsq_of(ap):
                return xsq_tiles[ap // PPT], (ap % PPT) * BL

            xsq_done = [0]  # pairs squared so far

            def ensure_xsq(up_to_pair):
                while xsq_done[0] < min(up_to_pair, A // 2):
                    ap = xsq_done[0]
                    xtt, xoff = xt_of(ap)
                    xsqt, _ = xsq_of(ap)
                    sl = slice(xoff, xoff + BL)
                    nc.gpsimd.tensor_mul(xsqt[:, sl], xtt[:, sl], xtt[:, sl])
                    xsq_done[0] += 1

            c4_of_group: dict = {}

            def group_c4(g):
                if g not in c4_of_group:
                    c4_of_group[g] = psum_c.tile([128, BL], f32, name="c4")
                return c4_of_group[g]

            # unit type for path3 (pair, h): True -> GpSimd second mul
            unit_gps = {}
            cnt = 0
            for ap in range(A // 2):
                for h in (0, 1):
                    use = h == 1 and ap >= (A // 2) - NGPS and cnt < NGPS
                    if use:
                        cnt += 1
                    unit_gps[(ap, h)] = use

            def stage_p3(g, h):
                """Path3 stage for group g (pairs 2g, 2g+1), chunk h."""
                pairs = (2 * g, 2 * g + 1)

                def gathers():
                    outs = []
                    for ap in pairs:
                        xtt, xoff = xt_of(ap)
                        gb = psum_g.tile([128, 2 * BL], f32, tag="gath",
                                         name="gb")
                        ga = psum_g.tile([128, 2 * BL], f32, tag="gath",
                                         name="ga")
                        gc = psum_g.tile([128, 2 * BL], f32, tag="gath",
                                         name="gc")
                        for po, gs in ((0, slice(0, BL)),
                                       (64, slice(BL, 2 * BL))):
                            rs = slice(po, po + 64)
                            xa = xtt[rs, xoff : xoff + BL]
                            nc.tensor.matmul(
                                gb[:, gs],
                                sel3[rs, 256 + 128 * h : 384 + 128 * h],
                                xa, start=True, stop=True,
                                tile_position=(po, 0),
                            )
                            nc.tensor.matmul(
                                ga[:, gs],
                                sel3[rs, 128 * h : 128 * h + 128],
                                xa, start=True, stop=True,
                                tile_position=(po, 0),
                            )
                            nc.tensor.matmul(
                                gc[:, gs],
                                sel3[rs, 512 + 128 * h : 640 + 128 * h],
                                xa, start=True, stop=True,
                                tile_position=(po, 0),
                            )
                        outs.append((gb, ga, gc))
                    return outs

                def chain(outs):
                    x3s = []
                    for ap, (gb, ga, gc) in zip(pairs, outs):
                        gb_sb = work.tile([128, 2 * BL], bf16, tag="gsb",
                                          bufs=3, name="gb_sb")
                        nc.scalar.copy(gb_sb[:], gb[:])
                        t_sb = work.tile([128, 2 * BL], bf16, tag="tprod",
                                         bufs=3, name="t_sb")
                        nc.vector.tensor_mul(t_sb[:], ga[:], gb_sb[:])
                        x3 = work.tile([128, 2 * BL], bf16, tag="xprod",
                                       bufs=6, name="x3")
                        if unit_gps[(ap, h)]:
                            gc_sb = work.tile([128, 2 * BL], bf16, tag="gcsb",
                                              bufs=3, name="gc_sb")
                            nc.scalar.copy(gc_sb[:], gc[:])
                            nc.gpsimd.tensor_mul(x3[:], gc_sb[:], t_sb[:])
                        else:
                            nc.vector.tensor_mul(x3[:], gc[:], t_sb[:])
                        x3s.append(x3)
                    return x3s

                def contracts(x3s):
                    c4 = group_c4(g)
                    for ap, x3 in zip(pairs, x3s):
                        ae, ao = 2 * ap, 2 * ap + 1
                        je, jo = ae % AG, ao % AG
                        nc.tensor.matmul(
                            c4[32 * je : 32 * je + 32, :],
                            wg3[:, ae * 64 + 32 * h : ae * 64 + 32 * h + 32],
                            x3[:, 0:BL],
                            start=(h == 0), stop=False,
                            tile_position=(0, 32 * je),
                        )
                        nc.tensor.matmul(
                            c4[32 * jo : 32 * jo + 32, :],
                            wg3[:, ao * 64 + 32 * h : ao * 64 + 32 * h + 32],
                            x3[:, BL : 2 * BL],
                            start=(h == 0), stop=False,
                            tile_position=(0, 32 * jo),
                        )

                return gathers, chain, contracts

            def stage_p2(g):
                """Path2 + path1 + finalize stage for group g."""
                pairs = (2 * g, 2 * g + 1)

                def gathers():
                    outs = []
                    for ap in pairs:
                        xtt, xoff = xt_of(ap)
                        s_pair = psum_g.tile([128, 2 * BL], f32, tag="gath",
                                             name="s_pair")
                        for po, gs in ((0, slice(0, BL)),
                                       (64, slice(BL, 2 * BL))):
                            nc.tensor.matmul(
                                s_pair[:, gs], sel2[po : po + 64, :],
                                xtt[po : po + 64, xoff : xoff + BL],
                                start=True, stop=True, tile_position=(po, 0),
                            )
                        outs.append(s_pair)
                    return outs

                def chain(outs):
                    s2s = []
                    for s_pair in outs:
                        s2 = work.tile([128, 2 * BL], bf16, tag="s2",
                                       bufs=4, name="s2")
                        nc.scalar.square(s2[:], s_pair[:])
                        s2s.append(s2)
                    return s2s

                def contracts(s2s):
                    c4 = group_c4(g)
                    # wg2 contracts: 4x M=32, one per column group
                    for ap, s2 in zip(pairs, s2s):
                        ae, ao = 2 * ap, 2 * ap + 1
                        je, jo = ae % AG, ao % AG
                        nc.tensor.matmul(
                            c4[32 * je : 32 * je + 32, :],
                            wg2[:, ae * 32 : (ae + 1) * 32], s2[:, 0:BL],
                            start=False, stop=False,
                            tile_position=(0, 32 * je),
                        )
                        nc.tensor.matmul(
                            c4[32 * jo : 32 * jo + 32, :],
                            wg2[:, ao * 32 : (ao + 1) * 32],
                            s2[:, BL : 2 * BL],
                            start=False, stop=False,
                            tile_position=(0, 32 * jo),
                        )
                    # a1w (path1) then sqw (path2 correction): M=64 each,
                    # 2-way col tiled across the two pairs
                    for ap in pairs:
                        je = (2 * ap) % AG
                        xtt, xoff = xt_of(ap)
                        nc.tensor.matmul(
                            c4[32 * je : 32 * je + 64, :],
                            a1w[:, ap * 64 : (ap + 1) * 64],
                            xtt[:, xoff : xoff + BL],
                            start=False, stop=False,
                            tile_position=(0, 32 * je),
                        )
                    for ap in pairs:
                        je = (2 * ap) % AG
                        xsqt, xoff = xsq_of(ap)
                        nc.tensor.matmul(
                            c4[32 * je : 32 * je + 64, :],
                            sqw[:, ap * 64 : (ap + 1) * 64],
                            xsqt[:, xoff : xoff + BL],
                            start=False, stop=True,
                            tile_position=(0, 32 * je),
                        )
                    # finalize: collapse e with y-weights, write the
                    # block-sum into rows 0:4 of this group's (now fully
                    # consumed) c4 bank, copy out, DMA.
                    v = work.tile([128, BL], bf16, tag="vmul", bufs=2,
                                  name="v")
                    nc.vector.tensor_mul(v[:], c4[:], y4[:])
                    nc.tensor.matmul(
                        c4[0:4, :], sig[:, :], v[:], start=True, stop=True,
                        skip_group_check=True,
                    )
                    o4_sb = work.tile([AG, BL], f32, tag="osb", bufs=2,
                                      name="o4_sb")
                    nc.scalar.copy(o4_sb[:], c4[0:4, :])
                    nc.sync.dma_start(
                        out=out_d[:, g * BL : (g + 1) * BL], in_=o4_sb[:]
                    )

                return gathers, chain, contracts

            # ---- emission: software pipeline over stages ----
            stages = []
            for g in range(NG):
                stages.append((g, stage_p3(g, 0)))
                stages.append((g, stage_p3(g, 1)))
                stages.append((g, stage_p2(g)))

            LAG = 2  # stages of lookahead between gathers and contracts
            ensure_xsq(4)
            pending = []
            for g, (gathers, chain, contracts) in stages:
                ensure_xsq(2 * (g + 2))
                gout = gathers()
                cout = chain(gout)
                pending.append((cout, contracts))
                if len(pending) > LAG:
                    out0, k0 = pending.pop(0)
                    k0(out0)
            for out0, k0 in pending:
                k0(out0)

    nc.compile()
    return nc


def _host_prepare(x, y, w1, w2, w3, coeff1, coeff2, coeff3, idx):
    """Build per-core input maps (bf16 numpy via ml_dtypes)."""
    import ml_dtypes

    bf = ml_dtypes.bfloat16

    (i1, l1, k1, q1, i2, j2, l2, m2, k2, q2,
     i3, j3, f3, l3, m3, g3, k3, q3) = idx

    xf = np.ascontiguousarray(x.reshape(B, A, L * M), dtype=np.float32)
    c1 = i1 * M + l1
    c2a = i2 * M + l2
    c2b = j2 * M + m2
    c3a = i3 * M + l3
    c3b = j3 * M + m3
    c3c = f3 * M + g3

    # sel2: 2-hot (or single 2) columns for s = x[c2a] + x[c2b]
    sel2h = np.zeros((64, 128), dtype=np.float32)
    np.add.at(sel2h, (c2a, np.arange(P2)), 1.0)
    np.add.at(sel2h, (c2b, np.arange(P2)), 1.0)
    sel2 = np.concatenate([sel2h, sel2h], axis=0).astype(bf)

    # sel3 one-hot gathers: cols [h*128 + p] for factor a, +256 b, +512 c
    sel3h = np.zeros((64, 768), dtype=np.float32)
    pa = np.arange(P3)
    sel3h[c3a, pa] = 1.0
    sel3h[c3b, 256 + pa] = 1.0
    sel3h[c3c, 512 + pa] = 1.0
    sel3 = np.concatenate([sel3h, sel3h], axis=0).astype(bf)

    def blockdiag_pairs(m3):
        """[64, A, 32] folded stationary -> per-pair block-diagonal
        [128, (A//2)*64]: rows 0-63 even-a block in cols 0:32, rows
        64-127 odd-a block in cols 32:64 of each pair's 64-col slot."""
        out = np.zeros((128, (A // 2) * 64), dtype=np.float32)
        for ap in range(A // 2):
            out[0:64, ap * 64 : ap * 64 + 32] = m3[:, 2 * ap, :]
            out[64:128, ap * 64 + 32 : ap * 64 + 64] = m3[:, 2 * ap + 1, :]
        return out

    # a1w[c, a*32+e] = sum_{p: c1[p]=c} coeff1[p] * w1[e, k1[p], q1[p], a]
    W1g = (w1[:, k1, q1, :] * coeff1[None, :, None]).transpose(1, 2, 0)  # [P1,A,E]
    a1w3 = np.zeros((64, A, 32), dtype=np.float32)
    np.add.at(a1w3[:, :, :E], c1, W1g)
    a1w = blockdiag_pairs(a1w3).astype(bf)

    # path2: wg2 = W2g/2 (contract on s^2); sqw = -fold(W2g/2) (on xsq)
    W2g = (w2[:, k2, q2, :] * coeff2[None, :, None]).transpose(1, 2, 0)  # [P2,A,E]
    wg2_3 = np.zeros((P2, A, 32), dtype=np.float32)
    wg2_3[:, :, :E] = 0.5 * W2g
    wg2 = wg2_3.reshape(P2, A * 32).astype(bf)
    sqw3 = np.zeros((64, A, 32), dtype=np.float32)
    np.add.at(sqw3[:, :, :E], c2a, -0.5 * W2g)
    np.add.at(sqw3[:, :, :E], c2b, -0.5 * W2g)
    sqw = blockdiag_pairs(sqw3).astype(bf)

    # path3 contract weights: col (a*64 + 32h + e), rows = chunk paths
    W3g = (w3[:, k3, q3, :] * coeff3[None, :, None]).transpose(1, 2, 0)  # [P3,A,E]
    wg3_3 = np.zeros((128, A, 64), dtype=np.float32)
    wg3_3[:, :, 0:E] = W3g[:128]
    wg3_3[:, :, 32 : 32 + E] = W3g[128:]
    wg3 = wg3_3.reshape(128, A * 64).astype(bf)

    sig = np.zeros((128, 4), dtype=np.float32)
    for j in range(AG):
        sig[32 * j : 32 * j + E, j] = 1.0
    sig = sig.astype(bf)

    in_maps = []
    for k in range(NCORES):
        xb = xf[k * BL : (k + 1) * BL]  # [BL, A, 64]
        xtf = xb.transpose(2, 1, 0)  # [c, a, b]
        xt = np.empty((128, (A // 2) * BL), dtype=np.float32)
        xt[:64] = np.ascontiguousarray(xtf[:, 0::2, :]).reshape(64, (A // 2) * BL)
        xt[64:] = np.ascontiguousarray(xtf[:, 1::2, :]).reshape(64, (A // 2) * BL)
        yb = np.asarray(y[k * BL : (k + 1) * BL], dtype=np.float32)  # [BL, E]
        y4 = np.zeros((128, BL), dtype=np.float32)
        for j in range(AG):
            y4[32 * j : 32 * j + E, :] = yb.T
        in_maps.append(
            {
                "xt": xt.astype(bf), "y4": y4.astype(bf), "sel2": sel2,
                "sel3": sel3, "a1w": a1w, "sqw": sqw, "wg2": wg2,
                "wg3": wg3, "sig": sig,
            }
        )
    return in_maps


def _run(inputs: dict, trace: bool = False):
    from concourse.bass_utils import run_bass_kernel_spmd

    if "nc" not in _CACHE:
        _CACHE["nc"] = _build_module()
    nc = _CACHE["nc"]

    idx = tuple(
        np.asarray(inputs[k], dtype=np.int64)
        for k in ("i1", "l1", "k1", "q1", "i2", "j2", "l2", "m2", "k2", "q2",
                  "i3", "j3", "f3", "l3", "m3", "g3", "k3", "q3")
    )
    in_maps = _host_prepare(
        np.asarray(inputs["x"], np.float32),
        np.asarray(inputs["y"], np.float32),
        np.asarray(inputs["w1"], np.float32),
        np.asarray(inputs["w2"], np.float32),
        np.asarray(inputs["w3"], np.float32),
        np.asarray(inputs["coeff1"], np.float32),
        np.asarray(inputs["coeff2"], np.float32),
        np.asarray(inputs["coeff3"], np.float32),
        idx,
    )

    res = run_bass_kernel_spmd(nc, in_maps, core_ids=list(range(NCORES)), trace=trace)

    out = np.empty((B, A), dtype=np.float32)
    for k in range(NCORES):
        o = res.results[k]["out"]  # [4, NG*BL]
        o = o.reshape(AG, NG, BL)  # [j, g, b]
        t_core = o.transpose(1, 0, 2).reshape(A, BL)  # [a, b]
        out[k * BL : (k + 1) * BL, :] = t_core.T
    return out, res


def kernel(**inputs) -> np.ndarray:
    out, _ = _run(inputs, trace=False)
    return out
